# revision 11
# baseline (speedup 1.0000x reference)
"""PeakDetectionLoss on 8 Trainium2 cores.

Sharding: time axis split into 8 segments (one per core), all 10 signal rows
on every core; host pre-pads halos so the width-11 sliding max needs no
device halo exchange. The sliding-max chain runs flat over row pairs in a
recycled SBUF arena; per-row stats accumulate on ScalarE/accumulators; the
cross-core stats AllReduce is split in two (rows 0-7 / 8-9) so the amplitude
filter + pairwise gap tree for rows 0-7 execute while the second AllReduce
is in flight. Gap reciprocals are deferred into one fast custom-DVE op; the
tree stops at 16 sub-strips per partition-row and the host stitches them.
"""
import os
import sys

for _p in ("/opt/trn_rl_repo", "/root/.axon_site/_ro/trn_rl_repo"):
    if _p not in sys.path:
        sys.path.append(_p)

import numpy as np

N = 5
L = 2097152
C = 8
SEG = L // C            # 262144
P = 128
PW = SEG // P           # 2048
NB = PW // 4            # 512
TILE_W = PW + 10        # 2058
R = 2 * N               # 10 rows per core
BIG = np.float32(1.0e30)
TW = 16                 # tree stops at 16 sub-strips per partition-row
NG = NB // 2 + NB // 4 + NB // 8 + NB // 16 + NB // 32   # 496 gap slots/row
SUMW = 2 * R * TW + 2 * R   # 340 summ cols

_STATE = {}


def _build_program():
    from concourse import bacc, tile, mybir
    from concourse.alu_op_type import AluOpType as op

    f32 = mybir.dt.float32
    nc = bacc.Bacc("TRN2", target_bir_lowering=False, debug=False, num_devices=C)

    xin = nc.dram_tensor("xin", [R, P, TILE_W], f32, kind="ExternalInput")
    bidx = nc.dram_tensor("bidx", [P, NB], f32, kind="ExternalInput")
    summ = nc.dram_tensor("summ", [P, SUMW], f32, kind="ExternalOutput")

    W = TILE_W              # 2058 pitch inside pair regions
    AR_W = 6 * 2 * W        # arena: 6 regions of [2, 2058]

    with tile.TileContext(nc) as tc:
        with (
            tc.tile_pool(name="sb", bufs=1) as sb,
            tc.tile_pool(name="dram", bufs=1, space="DRAM") as dram,
            tc.tile_pool(name="ps", bufs=1, space="PSUM") as ps,
        ):
            arena = sb.tile([P, AR_W], f32, tag="arena")
            bidx_sb = sb.tile([P, NB], f32, tag="bidx")
            n1_all = sb.tile([P, R * NB], f32, tag="n1_all")
            bn_all = sb.tile([P, R * NB], f32, tag="bn_all")
            B4cs = [sb.tile([P, 2, NB], f32, tag=f"B4c{s}", name=f"B4c{s}")
                    for s in (0, 1)]
            h = sb.tile([P, 2 * R * NB], f32, tag="h")
            # statsA rows 0-7: npk [0:8], SxA [8:16], SxB [16:24], sv [24:32]
            # statsB rows 8-9: npk [0:2], SxA [2:4], SxB [4:6], sv [6:8]
            statsA = sb.tile([P, 32], f32, tag="statsA")
            statsB = sb.tile([P, 8], f32, tag="statsB")
            jk = sb.tile([P, PW // 2], f32, tag="jk")
            summ_sb = sb.tile([P, SUMW], f32, tag="summ_sb")
            ones = sb.tile([P, 1], f32, tag="ones")
            ones_b = sb.tile([1, P], f32, tag="ones_b")
            big = sb.tile([P, 1], f32, tag="big")
            arsb1 = sb.tile([1, 32], f32, tag="arsb1")
            arst1 = sb.tile([1, 32], f32, tag="arst1")
            arsb2 = sb.tile([1, 8], f32, tag="arsb2")
            arst2 = sb.tile([1, 8], f32, tag="arst2")
            trec = sb.tile([1, R], f32, tag="trec")
            tsx = sb.tile([1, R], f32, tag="tsx")
            tmean = sb.tile([1, R], f32, tag="tmean")
            tthr = sb.tile([1, R], f32, tag="tthr")
            tbc = sb.tile([P, R], f32, tag="tbc")

            ar_in1 = dram.tile([1, 32], f32)
            ar_out1 = dram.tile([1, 32], f32)
            ar_in2 = dram.tile([1, 8], f32)
            ar_out2 = dram.tile([1, 8], f32)
            psumA = ps.tile([1, 32], f32)
            psumB = ps.tile([1, 8], f32)
            psb1 = ps.tile([P, 8], f32)
            psb2 = ps.tile([P, 2], f32)

            xin_ap = xin.ap()
            nc.vector.memset(ones, 1.0)
            nc.vector.memset(ones_b, 1.0)
            nc.vector.memset(big, float(BIG))

            # ---- phase A regions: slot s in {0,1}; 3 region kinds.
            # Chain ops run FLAT over both rows of a pair (junction garbage
            # is never read: all downstream views are per-row).
            def regf(i):  # i in 0..5 -> [P, 2*W] flat view
                return arena[:, i * 2 * W:(i + 1) * 2 * W]

            def reg3(i):  # same region as [P, 2, W]
                return regf(i).rearrange("p (r w) -> p r w", w=W)

            xtf = [regf(0), regf(1)]
            cbf = [regf(2), regf(3)]    # M2 / M8 / m1
            ccf = [regf(4), regf(5)]    # M4 / Wt
            xt3 = [reg3(0), reg3(1)]
            cb3 = [reg3(2), reg3(3)]
            cc3 = [reg3(4), reg3(5)]

            FW = 2 * W                  # 4116
            n1v = n1_all.rearrange("p (r b) -> p r b", b=NB)
            bnv = bn_all.rearrange("p (r b) -> p r b", b=NB)
            hv = h.rearrange("p (a r b) -> p a r b", a=2, r=R)
            bidx2 = bidx_sb.unsqueeze(1).broadcast_to([P, 2, NB])

            def dma_pair(p):
                s = p % 2
                for j in (0, 1):
                    half = W // 2
                    nc.sync.dma_start(
                        xt3[s][:, j, 0:half], xin_ap[2 * p + j][:, 0:half])
                    nc.sync.dma_start(
                        xt3[s][:, j, half:W], xin_ap[2 * p + j][:, half:W])

            def stage(p, k):
                s = p % 2
                xt, cb, cc = xtf[s], cbf[s], ccf[s]
                m13 = cb3[s][:, :, 0:PW]
                rows = slice(2 * p, 2 * p + 2)
                if k == 0:
                    nc.vector.tensor_tensor(
                        out=cb[:, 0:FW - 1], in0=xt[:, 0:FW - 1],
                        in1=xt[:, 1:FW], op=op.max)          # M2
                elif k == 1:
                    nc.vector.tensor_tensor(
                        out=cc[:, 0:FW - 3], in0=cb[:, 0:FW - 3],
                        in1=cb[:, 2:FW - 1], op=op.max)      # M4
                elif k == 2:
                    # aligned block-4 max copied out on DVE (same engine as
                    # Wt's overwrite: in-order, no cross-engine semaphore)
                    nc.vector.tensor_scalar(
                        out=B4cs[s], in0=cc3[s][:, :, 5:2052:4], scalar1=0.0,
                        scalar2=None, op0=op.add)
                    nc.vector.tensor_tensor(
                        out=cb[:, 0:FW - 7], in0=cc[:, 0:FW - 7],
                        in1=cc[:, 4:FW - 3], op=op.max)      # M8
                elif k == 3:
                    nc.vector.tensor_tensor(
                        out=cc[:, 0:FW - 10], in0=cb[:, 0:FW - 10],
                        in1=cb[:, 3:FW - 7], op=op.max)      # Wt
                elif k == 4:
                    # m1 = (x == window max), per row so the accumulator
                    # yields npk for free
                    HW2 = PW // 2
                    for j in (0, 1):
                        r_ = 2 * p + j
                        st, c0 = (statsA, r_) if r_ < 8 else (statsB, r_ - 8)
                        nc.vector.scalar_tensor_tensor(
                            out=cb3[s][:, j, 0:PW],
                            in0=xt3[s][:, j, 5:5 + PW],
                            scalar=0.0, op0=op.bypass,
                            in1=cc3[s][:, j, 0:PW], op1=op.is_ge,
                            accum_out=st[:, c0:c0 + 1])
                        # per-row signal sums on ScalarE in two halves into a
                        # junk tile (no write into any DVE-owned region)
                        sxa = (statsA[:, 8 + r_:9 + r_] if r_ < 8
                               else statsB[:, 2 + c0:3 + c0])
                        sxb = (statsA[:, 16 + r_:17 + r_] if r_ < 8
                               else statsB[:, 4 + c0:5 + c0])
                        nc.scalar.activation(
                            out=jk, in_=xt3[s][:, j, 5:5 + HW2],
                            func=mybir.ActivationFunctionType.Copy,
                            accum_out=sxa)
                        nc.scalar.activation(
                            out=jk, in_=xt3[s][:, j, 5 + HW2:5 + PW],
                            func=mybir.ActivationFunctionType.Copy,
                            accum_out=sxb)
                elif k == 5:
                    nc.vector.tensor_reduce(
                        out=n1v[:, rows],
                        in_=m13.rearrange("p r (b k) -> p r b k", k=4),
                        axis=mybir.AxisListType.X, op=op.add)
                elif k == 6:
                    # bn = B4*n1 -> bn_all, per row so the accumulator
                    # yields sv for free (no separate reduce)
                    for j in (0, 1):
                        r_ = 2 * p + j
                        svc = (statsA[:, 24 + r_:25 + r_] if r_ < 8
                               else statsB[:, 6 + r_ - 8:7 + r_ - 8])
                        nc.vector.scalar_tensor_tensor(
                            out=bnv[:, r_], in0=B4cs[s][:, j], scalar=0.0,
                            op0=op.bypass, in1=n1v[:, r_], op1=op.mult,
                            accum_out=svc)
                elif k == 7:
                    pass
                elif k == 8:
                    # peak slot in block: p1 -> h[pos], p2 -> h[neg] (stash)
                    nc.vector.scalar_tensor_tensor(
                        out=hv[:, 0, rows], in0=m13[:, :, 2:PW:4], scalar=2.0,
                        op0=op.mult, in1=m13[:, :, 1:PW:4], op1=op.add)
                elif k == 9:
                    nc.vector.scalar_tensor_tensor(
                        out=hv[:, 1, rows], in0=m13[:, :, 3:PW:4], scalar=3.0,
                        op0=op.mult, in1=hv[:, 0, rows], op1=op.add)
                elif k == 10:
                    nc.vector.tensor_tensor(
                        out=hv[:, 0, rows], in0=hv[:, 1, rows], in1=bidx2,
                        op=op.add)
                    nc.scalar.mul(hv[:, 1, rows], hv[:, 0, rows], -1.0)

            NSTAGE = 11
            # arena reuse while pair 4 (slot 0) is in flight: slot-1 xt/cc
            # regions (1, 5) are free; region 3 still holds pair 3's m1
            # (read post-AR2-kick by its deferred p-ops).

            # first pair's four half-row DMAs on four separate queues
            half = W // 2
            for j, eng in ((0, nc.sync), (1, nc.scalar)):
                eng.dma_start(xt3[0][:, j, 0:half], xin_ap[j][:, 0:half])
            for j, eng in ((0, nc.gpsimd), (1, nc.sync)):
                eng.dma_start(xt3[0][:, j, half:W], xin_ap[j][:, half:W])
            dma_pair(1)
            nc.sync.dma_start(bidx_sb, bidx.ap())
            # pair 0's M2 runs per row: row 0 starts as soon as its DMA
            # lands instead of waiting for both rows (junction column is
            # garbage either way; downstream views are per-row)
            nc.vector.tensor_tensor(
                out=cb3[0][:, 0, 0:half - 1], in0=xt3[0][:, 0, 0:half - 1],
                in1=xt3[0][:, 0, 1:half], op=op.max)
            nc.vector.tensor_tensor(
                out=cb3[0][:, 0, half - 1:2057],
                in0=xt3[0][:, 0, half - 1:2057],
                in1=xt3[0][:, 0, half:2058], op=op.max)
            nc.vector.tensor_tensor(
                out=cb3[0][:, 1, 0:2057], in0=xt3[0][:, 1, 0:2057],
                in1=xt3[0][:, 1, 1:2058], op=op.max)
            for k in range(NSTAGE):
                if k > 0:
                    stage(0, k)
                stage(1, k)
                if k == 4:
                    dma_pair(2)
                elif k == 6:
                    dma_pair(3)
            # pairs 2,3 through m1/Sx; pair 4's DMA once slot-0 xt is free
            for k in range(5):
                stage(2, k)
                stage(3, k)
            dma_pair(4)
            for k in range(5, 8):
                stage(2, k)
                stage(3, k)
            # pair 2's p-ops must precede pair 4's slot-0 reuse; pair 3's
            # p-ops are deferred past the AR2 kick as gap fodder
            stage(2, 8)
            stage(2, 9)
            stage(2, 10)

            # ---- AllReduce 1: rows 0..7 stats ----
            nc.tensor.matmul(
                out=psumA[0:1, :], lhsT=ones, rhs=statsA,
                start=True, stop=True)
            nc.scalar.copy(arst1, psumA[0:1, :])
            nc.sync.dma_start(ar_in1, arst1)
            nc.gpsimd.collective_compute(
                "AllReduce", op.add, replica_groups=[list(range(C))],
                ins=[ar_in1.opt()], outs=[ar_out1.opt()])
            nc.sync.dma_start(arsb1, ar_out1)

            # ---- pair 4 chain (DVE-internal deps are free) ----
            stage(4, 0)
            stage(4, 1)
            stage(4, 2)
            stage(4, 3)
            stage(4, 4)
            stage(4, 5)
            stage(4, 6)
            stage(4, 7)

            # ---- AllReduce 2: rows 8..9 stats (pipelines behind AR1) ----
            nc.tensor.matmul(
                out=psumB[0:1, :], lhsT=ones, rhs=statsB,
                start=True, stop=True)
            nc.scalar.copy(arst2, psumB[0:1, :])
            nc.sync.dma_start(ar_in2, arst2)
            nc.gpsimd.collective_compute(
                "AllReduce", op.add, replica_groups=[list(range(C))],
                ins=[ar_in2.opt()], outs=[ar_out2.opt()])
            nc.sync.dma_start(arsb2, ar_out2)

            # threshold-independent fodder while AR1 lands / AR2 flies
            stage(3, 8)
            stage(3, 9)
            stage(3, 10)
            stage(4, 8)
            stage(4, 9)
            stage(4, 10)


            # ---- threshold 1 (rows 0..7): t = Sx/(2L) + 0.5*sv/npk ----
            nc.vector.reciprocal(out=trec[0:1, 0:8], in_=arsb1[0:1, 0:8])
            nc.vector.scalar_tensor_tensor(
                out=tmean[0:1, 0:8], in0=trec[0:1, 0:8], scalar=0.5,
                op0=op.mult, in1=arsb1[0:1, 24:32], op1=op.mult)
            nc.vector.tensor_tensor(
                out=tsx[0:1, 0:8], in0=arsb1[0:1, 8:16],
                in1=arsb1[0:1, 16:24], op=op.add)
            nc.vector.scalar_tensor_tensor(
                out=tthr[0:1, 0:8], in0=tsx[0:1, 0:8], scalar=0.5 / L,
                op0=op.mult, in1=tmean[0:1, 0:8], op1=op.add)
            # broadcast across partitions via TensorE (gpsimd holds the ARs)
            nc.tensor.matmul(
                out=psb1, lhsT=ones_b, rhs=tthr[0:1, 0:8],
                start=True, stop=True)
            nc.scalar.copy(tbc[:, 0:8], psb1)

            # ---- phase B rows 0..7 (overlaps AR2 flight) ----
            notv = arena[:, 0:R * NB].rearrange("p (r b) -> p r b", b=NB)
            # per-row is_le with accumulator: notv mask and the per-row
            # below-threshold count in one pass. The signal is standardized,
            # so t = Sx/(2L) + mean_pk/2 > 0 always and bn <= t covers empty
            # blocks (bn = 0) too: no -BIG sentinel tensor is needed.
            for r_ in range(8):
                nc.vector.scalar_tensor_tensor(
                    out=notv[:, r_], scalar=0.0, op0=op.bypass,
                    in0=bn_all[:, r_ * NB:(r_ + 1) * NB],
                    in1=tbc[:, r_:r_ + 1].broadcast_to([P, NB]),
                    op1=op.is_le,
                    accum_out=summ_sb[:, SUMW - R + r_:SUMW - R + r_ + 1])
            nc.vector.copy_predicated(
                out=hv[:, :, 0:8],
                mask=notv[:, 0:8].bitcast(mybir.dt.int32).unsqueeze(1)
                .broadcast_to([P, 2, 8, NB]),
                data=big.broadcast_to([P, 2, 8, NB]))

            treeB = arena[:, 4 * W:4 * W + 2 * R * 256].rearrange(
                "p (a r c) -> p a r c", a=2, r=R)
            treeC = arena[:, 4 * W + 2 * R * 256:
                          4 * W + 2 * R * 256 + 2 * R * 128].rearrange(
                "p (a r c) -> p a r c", a=2, r=R)
            # gaps deferred: raw g values land in scratch, one fast recip after
            _soff = 4 * W + 2 * R * 256 + 2 * R * 128
            scratch = arena[:, _soff:_soff + R * NG].rearrange(
                "p (r c) -> p r c", c=NG)
            rbuf = arena[:, 0:R * NG].rearrange(
                "p (r c) -> p r c", c=NG)
            summ_h = summ_sb[:, 0:2 * R * TW].rearrange(
                "p (a r c) -> p a r c", a=2, r=R)

            def tree(rg):
                cur = hv[:, :, rg]
                nr = rg.stop - rg.start
                w = NB
                off = 0
                lvl = 0
                while w > TW:
                    w2 = w // 2
                    out_h = (summ_h[:, :, rg] if w2 == TW
                             else bufs_cycle[lvl % 2][:, :, rg, 0:w2])
                    nc.vector.tensor_tensor(
                        out=out_h, in0=cur[:, :, :, 0:w:2],
                        in1=cur[:, :, :, 1:w:2], op=op.min)
                    nc.vector.tensor_tensor(
                        out=scratch[:, rg, off:off + w2].unsqueeze(1),
                        in0=cur[:, 0:1, :, 1:w:2],
                        in1=cur[:, 1:2, :, 0:w:2],
                        op=op.add)
                    off += w2
                    cur = out_h
                    w = w2
                    lvl += 1

            bufs_cycle = [treeB, treeC]
            tree(slice(0, 8))

            # ---- threshold 2 lands mid-tree; broadcast overlaps recip ----
            nc.vector.reciprocal(out=trec[0:1, 8:10], in_=arsb2[0:1, 0:2])
            nc.vector.scalar_tensor_tensor(
                out=tmean[0:1, 8:10], in0=trec[0:1, 8:10], scalar=0.5,
                op0=op.mult, in1=arsb2[0:1, 6:8], op1=op.mult)
            nc.vector.tensor_tensor(
                out=tsx[0:1, 8:10], in0=arsb2[0:1, 2:4],
                in1=arsb2[0:1, 4:6], op=op.add)
            nc.vector.scalar_tensor_tensor(
                out=tthr[0:1, 8:10], in0=tsx[0:1, 8:10], scalar=0.5 / L,
                op0=op.mult, in1=tmean[0:1, 8:10], op1=op.add)
            nc.tensor.matmul(
                out=psb2, lhsT=ones_b, rhs=tthr[0:1, 8:10],
                start=True, stop=True)
            nc.scalar.copy(tbc[:, 8:10], psb2)

            nc.vector.reciprocal_approx_fast(
                out=rbuf[:, 0:8], in_=scratch[:, 0:8])
            nc.vector.tensor_reduce(
                out=summ_sb[:, 2 * R * TW:2 * R * TW + 8], in_=rbuf[:, 0:8],
                axis=mybir.AxisListType.X, op=op.add)
            # rows 0-7 output columns fly while rows 8-9 finish
            summ_ap = summ.ap()
            for c0, c1 in ((0, 8 * TW), (R * TW, R * TW + 8 * TW),
                           (2 * R * TW, 2 * R * TW + 8),
                           (SUMW - R, SUMW - 2)):
                nc.sync.dma_start(summ_ap[:, c0:c1], summ_sb[:, c0:c1])

            # ---- phase B rows 8..9 ----
            for r_ in (8, 9):
                nc.vector.scalar_tensor_tensor(
                    out=notv[:, r_], scalar=0.0, op0=op.bypass,
                    in0=bn_all[:, r_ * NB:(r_ + 1) * NB],
                    in1=tbc[:, r_:r_ + 1].broadcast_to([P, NB]),
                    op1=op.is_le,
                    accum_out=summ_sb[:, SUMW - R + r_:SUMW - R + r_ + 1])
            nc.vector.copy_predicated(
                out=hv[:, :, 8:10],
                mask=notv[:, 8:10].bitcast(mybir.dt.int32).unsqueeze(1)
                .broadcast_to([P, 2, 2, NB]),
                data=big.broadcast_to([P, 2, 2, NB]))
            tree(slice(8, 10))
            nc.vector.reciprocal_approx_fast(
                out=rbuf[:, 8:10], in_=scratch[:, 8:10])
            nc.vector.tensor_reduce(
                out=summ_sb[:, 2 * R * TW + 8:2 * R * TW + R],
                in_=rbuf[:, 8:10],
                axis=mybir.AxisListType.X, op=op.add)
            for c0, c1 in ((8 * TW, R * TW), ((R + 8) * TW, 2 * R * TW),
                           (2 * R * TW + 8, 2 * R * TW + R),
                           (SUMW - 2, SUMW)):
                nc.sync.dma_start(summ_ap[:, c0:c1], summ_sb[:, c0:c1])

    nc.compile()
    return nc


def _get_runner():
    """Build once; return fn(in_maps) -> list of per-core {name: np.ndarray}."""
    if "runner" in _STATE:
        return _STATE["runner"]

    import jax
    import jax.numpy as jnp
    from jax.sharding import Mesh, PartitionSpec
    from jax.experimental.shard_map import shard_map
    from concourse import bass2jax, mybir

    nc = _build_program()
    bass2jax.install_neuronx_cc_hook()

    partition_name = (
        nc.partition_id_tensor.name if nc.partition_id_tensor else None
    )
    in_names, out_names, out_avals, zero_outs = [], [], [], []
    for alloc in nc.m.functions[0].allocations:
        if not isinstance(alloc, mybir.MemoryLocationSet):
            continue
        name = alloc.memorylocations[0].name
        if alloc.kind == "ExternalInput":
            if name != partition_name:
                in_names.append(name)
        elif alloc.kind == "ExternalOutput":
            out_names.append(name)
            shape = tuple(alloc.tensor_shape)
            dtype = mybir.dt.np(alloc.dtype)
            out_avals.append(jax.core.ShapedArray(shape, dtype))
            zero_outs.append(np.zeros(shape, dtype))
    n_params = len(in_names)
    n_outs = len(out_avals)
    all_names = in_names + out_names
    if partition_name is not None:
        all_names = all_names + [partition_name]

    def _body(*args):
        operands = list(args)
        if partition_name is not None:
            operands.append(bass2jax.partition_id_tensor())
        outs = bass2jax._bass_exec_p.bind(
            *operands,
            out_avals=tuple(out_avals),
            in_names=tuple(all_names),
            out_names=tuple(out_names),
            lowering_input_output_aliases=(),
            sim_require_finite=False,
            sim_require_nnan=False,
            nc=nc,
        )
        return tuple(outs)

    devices = jax.devices()[:C]
    assert len(devices) == C, f"need {C} devices, have {len(jax.devices())}"
    mesh = Mesh(np.asarray(devices), ("core",))
    donate = tuple(range(n_params, n_params + n_outs))
    sharded = jax.jit(
        shard_map(
            _body, mesh=mesh,
            in_specs=(PartitionSpec("core"),) * (n_params + n_outs),
            out_specs=(PartitionSpec("core"),) * n_outs,
            check_rep=False,
        ),
        donate_argnums=donate,
        keep_unused=True,
    )

    def run(in_maps):
        concat_in = [
            np.concatenate([np.asarray(m[nm]) for m in in_maps], axis=0)
            for nm in in_names
        ]
        concat_zeros = [
            np.zeros((C * z.shape[0], *z.shape[1:]), z.dtype) for z in zero_outs
        ]
        out_arrs = sharded(*concat_in, *concat_zeros)
        return [
            {nm: np.asarray(out_arrs[i]).reshape(C, *out_avals[i].shape)[c]
             for i, nm in enumerate(out_names)}
            for c in range(C)
        ]

    run.in_names = in_names
    run.out_names = out_names
    run.sharded = sharded
    run.zero_outs = zero_outs
    _STATE["runner"] = run
    return run


def make_in_maps(rppg, ppg):
    sigs = np.concatenate(
        [np.asarray(ppg, np.float32).reshape(N, L),
         np.asarray(rppg, np.float32).reshape(N, L)], axis=0)
    padded = np.full((R, L + 10), -np.inf, np.float32)
    padded[:, 5:5 + L] = sigs
    win = np.lib.stride_tricks.sliding_window_view(padded, TILE_W, axis=1)
    in_maps = []
    prow = np.arange(P, dtype=np.float32)[:, None] * PW
    brow = np.arange(NB, dtype=np.float32)[None, :] * 4.0
    for c in range(C):
        xin_c = np.ascontiguousarray(win[:, c * SEG:c * SEG + SEG:PW, :])
        bidx_c = (np.float32(c * SEG) + prow + brow).astype(np.float32)
        in_maps.append({"xin": xin_c, "bidx": bidx_c})
    return in_maps


def stitch(results, fs):
    summ = np.stack([results[c]["summ"] for c in range(C)])  # [C, 128, 340]
    hr = np.zeros(R)
    for r in range(R):
        # sub-strips of TW blocks: [C, P, TW] in global time order
        f = summ[:, :, r * TW:(r + 1) * TW].reshape(-1).astype(np.float64)
        g = -summ[:, :, (R + r) * TW:(R + r + 1) * TW].reshape(-1).astype(
            np.float64)
        s = summ[:, :, 2 * R * TW + r].astype(np.float64).sum()
        n = (512.0 - summ[:, :, 2 * R * TW + R + r].astype(np.float64)).sum()
        ne = f < float(BIG) / 2
        fs_, gs_ = f[ne], g[ne]
        s += (1.0 / (fs_[1:] - gs_[:-1])).sum()
        hr[r] = 60.0 * float(fs) * s / (n - 1.0)
    return np.float32(np.mean(np.abs(hr[0:N] - hr[N:R]) / hr[0:N]))


def kernel(rppg, ppg, fs, epoch):
    run = _get_runner()
    results = run(make_in_maps(rppg, ppg))
    return stitch(results, fs)


# revision 12
# speedup vs baseline: 1.0063x; 1.0063x over previous
"""PeakDetectionLoss on 8 Trainium2 cores.

Sharding: time axis split into 8 segments (one per core), all 10 signal rows
on every core; host pre-pads halos so the width-11 sliding max needs no
device halo exchange. The sliding-max chain runs flat over row pairs in a
recycled SBUF arena; per-row stats accumulate on ScalarE/accumulators; the
cross-core stats AllReduce is split in two (rows 0-7 / 8-9) so the amplitude
filter + pairwise gap tree for rows 0-7 execute while the second AllReduce
is in flight. Gap reciprocals are deferred into one fast custom-DVE op; the
tree stops at 16 sub-strips per partition-row and the host stitches them.
"""
import os
import sys

for _p in ("/opt/trn_rl_repo", "/root/.axon_site/_ro/trn_rl_repo"):
    if _p not in sys.path:
        sys.path.append(_p)

import numpy as np

N = 5
L = 2097152
C = 8
SEG = L // C            # 262144
P = 128
PW = SEG // P           # 2048
NB = PW // 4            # 512
TILE_W = PW + 10        # 2058
R = 2 * N               # 10 rows per core
BIG = np.float32(1.0e30)
TW = 16                 # tree stops at 16 sub-strips per partition-row
NG = NB // 2 + NB // 4 + NB // 8 + NB // 16 + NB // 32   # 496 gap slots/row
SUMW = 2 * R * TW + 2 * R   # 340 summ cols

_STATE = {}


def _build_program():
    from concourse import bacc, tile, mybir
    from concourse.alu_op_type import AluOpType as op

    f32 = mybir.dt.float32
    nc = bacc.Bacc("TRN2", target_bir_lowering=False, debug=False, num_devices=C)

    xin = nc.dram_tensor("xin", [R, P, TILE_W], f32, kind="ExternalInput")
    bidx = nc.dram_tensor("bidx", [P, NB], f32, kind="ExternalInput")
    summ = nc.dram_tensor("summ", [P, SUMW], f32, kind="ExternalOutput")

    W = TILE_W              # 2058 pitch inside pair regions
    AR_W = 6 * 2 * W        # arena: 6 regions of [2, 2058]

    with tile.TileContext(nc) as tc:
        with (
            tc.tile_pool(name="sb", bufs=1) as sb,
            tc.tile_pool(name="dram", bufs=1, space="DRAM") as dram,
            tc.tile_pool(name="ps", bufs=1, space="PSUM") as ps,
        ):
            arena = sb.tile([P, AR_W], f32, tag="arena")
            bidx_sb = sb.tile([P, NB], f32, tag="bidx")
            n1_all = sb.tile([P, R * NB], f32, tag="n1_all")
            bn_all = sb.tile([P, R * NB], f32, tag="bn_all")
            B4cs = [sb.tile([P, 2, NB], f32, tag=f"B4c{s}", name=f"B4c{s}")
                    for s in (0, 1)]
            h = sb.tile([P, 2 * R * NB], f32, tag="h")
            # statsA rows 0-7: npk [0:8], SxA [8:16], SxB [16:24], sv [24:32]
            # statsB rows 8-9: npk [0:2], SxA [2:4], SxB [4:6], sv [6:8]
            statsA = sb.tile([P, 32], f32, tag="statsA")
            statsB = sb.tile([P, 8], f32, tag="statsB")
            jk = sb.tile([P, PW // 2], f32, tag="jk")
            summ_sb = sb.tile([P, SUMW], f32, tag="summ_sb")
            ones = sb.tile([P, 1], f32, tag="ones")
            ones_b = sb.tile([1, P], f32, tag="ones_b")
            big = sb.tile([P, 1], f32, tag="big")
            arsb1 = sb.tile([1, 32], f32, tag="arsb1")
            arst1 = sb.tile([1, 32], f32, tag="arst1")
            arsb2 = sb.tile([1, 8], f32, tag="arsb2")
            arst2 = sb.tile([1, 8], f32, tag="arst2")
            trec = sb.tile([1, R], f32, tag="trec")
            tsx = sb.tile([1, R], f32, tag="tsx")
            tmean = sb.tile([1, R], f32, tag="tmean")
            tthr = sb.tile([1, R], f32, tag="tthr")
            tbc = sb.tile([P, R], f32, tag="tbc")

            ar_in1 = dram.tile([1, 32], f32)
            ar_out1 = dram.tile([1, 32], f32)
            ar_in2 = dram.tile([1, 8], f32)
            ar_out2 = dram.tile([1, 8], f32)
            psumA = ps.tile([1, 32], f32)
            psumB = ps.tile([1, 8], f32)
            psb1 = ps.tile([P, 8], f32)
            psb2 = ps.tile([P, 2], f32)

            xin_ap = xin.ap()
            nc.vector.memset(ones, 1.0)
            nc.vector.memset(ones_b, 1.0)
            nc.vector.memset(big, float(BIG))

            # ---- phase A regions: slot s in {0,1}; 3 region kinds.
            # Chain ops run FLAT over both rows of a pair (junction garbage
            # is never read: all downstream views are per-row).
            def regf(i):  # i in 0..5 -> [P, 2*W] flat view
                return arena[:, i * 2 * W:(i + 1) * 2 * W]

            def reg3(i):  # same region as [P, 2, W]
                return regf(i).rearrange("p (r w) -> p r w", w=W)

            xtf = [regf(0), regf(1)]
            cbf = [regf(2), regf(3)]    # M2 / M8 / m1
            ccf = [regf(4), regf(5)]    # M4 / Wt
            xt3 = [reg3(0), reg3(1)]
            cb3 = [reg3(2), reg3(3)]
            cc3 = [reg3(4), reg3(5)]

            FW = 2 * W                  # 4116
            n1v = n1_all.rearrange("p (r b) -> p r b", b=NB)
            bnv = bn_all.rearrange("p (r b) -> p r b", b=NB)
            hv = h.rearrange("p (a r b) -> p a r b", a=2, r=R)
            bidx2 = bidx_sb.unsqueeze(1).broadcast_to([P, 2, NB])

            def dma_pair(p):
                s = p % 2
                for j in (0, 1):
                    half = W // 2
                    nc.sync.dma_start(
                        xt3[s][:, j, 0:half], xin_ap[2 * p + j][:, 0:half])
                    nc.sync.dma_start(
                        xt3[s][:, j, half:W], xin_ap[2 * p + j][:, half:W])

            def stage(p, k):
                s = p % 2
                xt, cb, cc = xtf[s], cbf[s], ccf[s]
                m13 = cb3[s][:, :, 0:PW]
                rows = slice(2 * p, 2 * p + 2)
                if k == 0:
                    nc.vector.tensor_tensor(
                        out=cb[:, 0:FW - 1], in0=xt[:, 0:FW - 1],
                        in1=xt[:, 1:FW], op=op.max)          # M2
                elif k == 1:
                    nc.vector.tensor_tensor(
                        out=cc[:, 0:FW - 3], in0=cb[:, 0:FW - 3],
                        in1=cb[:, 2:FW - 1], op=op.max)      # M4
                elif k == 2:
                    # aligned block-4 max copied out on DVE (same engine as
                    # Wt's overwrite: in-order, no cross-engine semaphore)
                    nc.vector.tensor_scalar(
                        out=B4cs[s], in0=cc3[s][:, :, 5:2052:4], scalar1=0.0,
                        scalar2=None, op0=op.add)
                    nc.vector.tensor_tensor(
                        out=cb[:, 0:FW - 7], in0=cc[:, 0:FW - 7],
                        in1=cc[:, 4:FW - 3], op=op.max)      # M8
                elif k == 3:
                    nc.vector.tensor_tensor(
                        out=cc[:, 0:FW - 10], in0=cb[:, 0:FW - 10],
                        in1=cb[:, 3:FW - 7], op=op.max)      # Wt
                elif k == 4:
                    # m1 = (x == window max), per row so the accumulator
                    # yields npk for free
                    HW2 = PW // 2
                    for j in (0, 1):
                        r_ = 2 * p + j
                        st, c0 = (statsA, r_) if r_ < 8 else (statsB, r_ - 8)
                        nc.vector.scalar_tensor_tensor(
                            out=cb3[s][:, j, 0:PW],
                            in0=xt3[s][:, j, 5:5 + PW],
                            scalar=0.0, op0=op.bypass,
                            in1=cc3[s][:, j, 0:PW], op1=op.is_ge,
                            accum_out=st[:, c0:c0 + 1])
                        # per-row signal sums on ScalarE in two halves into a
                        # junk tile (no write into any DVE-owned region)
                        sxa = (statsA[:, 8 + r_:9 + r_] if r_ < 8
                               else statsB[:, 2 + c0:3 + c0])
                        sxb = (statsA[:, 16 + r_:17 + r_] if r_ < 8
                               else statsB[:, 4 + c0:5 + c0])
                        nc.scalar.activation(
                            out=jk, in_=xt3[s][:, j, 5:5 + HW2],
                            func=mybir.ActivationFunctionType.Copy,
                            accum_out=sxa)
                        nc.scalar.activation(
                            out=jk, in_=xt3[s][:, j, 5 + HW2:5 + PW],
                            func=mybir.ActivationFunctionType.Copy,
                            accum_out=sxb)
                elif k == 5:
                    nc.vector.tensor_reduce(
                        out=n1v[:, rows],
                        in_=m13.rearrange("p r (b k) -> p r b k", k=4),
                        axis=mybir.AxisListType.X, op=op.add)
                elif k == 6:
                    # bn = B4*n1 -> bn_all, per row so the accumulator
                    # yields sv for free (no separate reduce)
                    for j in (0, 1):
                        r_ = 2 * p + j
                        svc = (statsA[:, 24 + r_:25 + r_] if r_ < 8
                               else statsB[:, 6 + r_ - 8:7 + r_ - 8])
                        nc.vector.scalar_tensor_tensor(
                            out=bnv[:, r_], in0=B4cs[s][:, j], scalar=0.0,
                            op0=op.bypass, in1=n1v[:, r_], op1=op.mult,
                            accum_out=svc)
                elif k == 7:
                    pass
                elif k == 8:
                    # peak slot in block: p1 -> h[pos], p2 -> h[neg] (stash)
                    nc.vector.scalar_tensor_tensor(
                        out=hv[:, 0, rows], in0=m13[:, :, 2:PW:4], scalar=2.0,
                        op0=op.mult, in1=m13[:, :, 1:PW:4], op1=op.add)
                elif k == 9:
                    nc.vector.scalar_tensor_tensor(
                        out=hv[:, 1, rows], in0=m13[:, :, 3:PW:4], scalar=3.0,
                        op0=op.mult, in1=hv[:, 0, rows], op1=op.add)
                elif k == 10:
                    nc.vector.tensor_tensor(
                        out=hv[:, 0, rows], in0=hv[:, 1, rows], in1=bidx2,
                        op=op.add)
                    nc.scalar.mul(hv[:, 1, rows], hv[:, 0, rows], -1.0)

            NSTAGE = 11
            # arena reuse while pair 4 (slot 0) is in flight: slot-1 xt/cc
            # regions (1, 5) are free; region 3 still holds pair 3's m1
            # (read post-AR2-kick by its deferred p-ops).
            B4m08 = arena[:, 2 * W:2 * W + 8 * NB]           # region 1
            aB08 = arena[:, 10 * W:10 * W + 8 * NB]          # region 5
            B4m89 = arena[:, 10 * W:10 * W + 2 * NB]         # region 5 (late)
            aB89 = arena[:, 10 * W + 2 * NB:10 * W + 4 * NB]

            # first pair's four half-row DMAs on four separate queues
            half = W // 2
            for j, eng in ((0, nc.sync), (1, nc.scalar)):
                eng.dma_start(xt3[0][:, j, 0:half], xin_ap[j][:, 0:half])
            for j, eng in ((0, nc.gpsimd), (1, nc.sync)):
                eng.dma_start(xt3[0][:, j, half:W], xin_ap[j][:, half:W])
            dma_pair(1)
            nc.sync.dma_start(bidx_sb, bidx.ap())
            # pair 0's M2 runs per row: row 0 starts as soon as its DMA
            # lands instead of waiting for both rows (junction column is
            # garbage either way; downstream views are per-row)
            nc.vector.tensor_tensor(
                out=cb3[0][:, 0, 0:half - 1], in0=xt3[0][:, 0, 0:half - 1],
                in1=xt3[0][:, 0, 1:half], op=op.max)
            nc.vector.tensor_tensor(
                out=cb3[0][:, 0, half - 1:2057],
                in0=xt3[0][:, 0, half - 1:2057],
                in1=xt3[0][:, 0, half:2058], op=op.max)
            nc.vector.tensor_tensor(
                out=cb3[0][:, 1, 0:2057], in0=xt3[0][:, 1, 0:2057],
                in1=xt3[0][:, 1, 1:2058], op=op.max)
            for k in range(NSTAGE):
                if k > 0:
                    stage(0, k)
                stage(1, k)
                if k == 4:
                    dma_pair(2)
                elif k == 6:
                    dma_pair(3)
            # pairs 2,3 through m1/Sx; pair 4's DMA once slot-0 xt is free
            for k in range(5):
                stage(2, k)
                stage(3, k)
            dma_pair(4)
            for k in range(5, 8):
                stage(2, k)
                stage(3, k)
            # pair 2's p-ops must precede pair 4's slot-0 reuse; pair 3's
            # p-ops are deferred past the AR2 kick as gap fodder
            stage(2, 8)
            stage(2, 9)
            stage(2, 10)

            # ---- AllReduce 1: rows 0..7 stats ----
            nc.tensor.matmul(
                out=psumA[0:1, :], lhsT=ones, rhs=statsA,
                start=True, stop=True)
            nc.scalar.copy(arst1, psumA[0:1, :])
            nc.sync.dma_start(ar_in1, arst1)
            nc.gpsimd.collective_compute(
                "AllReduce", op.add, replica_groups=[list(range(C))],
                ins=[ar_in1.opt()], outs=[ar_out1.opt()])
            nc.sync.dma_start(arsb1, ar_out1)

            # aB = n1*BIG - BIG on ScalarE (scale/bias); rows 0..7 early so
            # B4m chunks interleave with pair 4's chain
            nc.scalar.activation(
                out=aB08, in_=n1_all[:, 0:8 * NB],
                func=mybir.ActivationFunctionType.Copy,
                bias=float(-BIG), scale=float(BIG))

            # ---- pair 4 chain (DVE-internal deps are free) + fodder ----
            stage(4, 0)
            stage(4, 1)
            nc.vector.tensor_tensor(
                out=B4m08[:, 0:4 * NB], in0=bn_all[:, 0:4 * NB],
                in1=aB08[:, 0:4 * NB], op=op.add)
            stage(4, 2)
            nc.vector.tensor_tensor(
                out=B4m08[:, 4 * NB:8 * NB], in0=bn_all[:, 4 * NB:8 * NB],
                in1=aB08[:, 4 * NB:8 * NB], op=op.add)
            stage(4, 3)
            stage(4, 4)
            stage(4, 5)
            stage(4, 6)
            stage(4, 7)

            # ---- AllReduce 2: rows 8..9 stats (pipelines behind AR1) ----
            nc.tensor.matmul(
                out=psumB[0:1, :], lhsT=ones, rhs=statsB,
                start=True, stop=True)
            nc.scalar.copy(arst2, psumB[0:1, :])
            nc.sync.dma_start(ar_in2, arst2)
            nc.gpsimd.collective_compute(
                "AllReduce", op.add, replica_groups=[list(range(C))],
                ins=[ar_in2.opt()], outs=[ar_out2.opt()])
            nc.sync.dma_start(arsb2, ar_out2)

            # threshold-independent fodder while AR1 lands / AR2 flies
            stage(3, 8)
            stage(3, 9)
            stage(3, 10)
            stage(4, 8)
            stage(4, 9)
            stage(4, 10)
            nc.scalar.activation(
                out=aB89, in_=n1_all[:, 8 * NB:R * NB],
                func=mybir.ActivationFunctionType.Copy,
                bias=float(-BIG), scale=float(BIG))
            nc.vector.tensor_tensor(
                out=B4m89, in0=bn_all[:, 8 * NB:R * NB], in1=aB89, op=op.add)

            # ---- threshold 1 (rows 0..7): t = Sx/(2L) + 0.5*sv/npk ----
            nc.vector.reciprocal(out=trec[0:1, 0:8], in_=arsb1[0:1, 0:8])
            nc.vector.scalar_tensor_tensor(
                out=tmean[0:1, 0:8], in0=trec[0:1, 0:8], scalar=0.5,
                op0=op.mult, in1=arsb1[0:1, 24:32], op1=op.mult)
            nc.vector.tensor_tensor(
                out=tsx[0:1, 0:8], in0=arsb1[0:1, 8:16],
                in1=arsb1[0:1, 16:24], op=op.add)
            nc.vector.scalar_tensor_tensor(
                out=tthr[0:1, 0:8], in0=tsx[0:1, 0:8], scalar=0.5 / L,
                op0=op.mult, in1=tmean[0:1, 0:8], op1=op.add)
            # broadcast across partitions via TensorE (gpsimd holds the ARs)
            nc.tensor.matmul(
                out=psb1, lhsT=ones_b, rhs=tthr[0:1, 0:8],
                start=True, stop=True)
            nc.scalar.copy(tbc[:, 0:8], psb1)

            # ---- phase B rows 0..7 (overlaps AR2 flight) ----
            notv = arena[:, 0:R * NB].rearrange("p (r b) -> p r b", b=NB)
            # per-row is_le with accumulator: notv mask and the per-row
            # below-threshold count in one pass
            for r_ in range(8):
                nc.vector.scalar_tensor_tensor(
                    out=notv[:, r_], scalar=0.0, op0=op.bypass,
                    in0=B4m08[:, r_ * NB:(r_ + 1) * NB],
                    in1=tbc[:, r_:r_ + 1].broadcast_to([P, NB]),
                    op1=op.is_le,
                    accum_out=summ_sb[:, SUMW - R + r_:SUMW - R + r_ + 1])
            nc.vector.copy_predicated(
                out=hv[:, :, 0:8],
                mask=notv[:, 0:8].bitcast(mybir.dt.int32).unsqueeze(1)
                .broadcast_to([P, 2, 8, NB]),
                data=big.broadcast_to([P, 2, 8, NB]))

            treeB = arena[:, 4 * W:4 * W + 2 * R * 256].rearrange(
                "p (a r c) -> p a r c", a=2, r=R)
            treeC = arena[:, 4 * W + 2 * R * 256:
                          4 * W + 2 * R * 256 + 2 * R * 128].rearrange(
                "p (a r c) -> p a r c", a=2, r=R)
            # gaps deferred: raw g values land in scratch, one fast recip after
            _soff = 4 * W + 2 * R * 256 + 2 * R * 128
            scratch = arena[:, _soff:_soff + R * NG].rearrange(
                "p (r c) -> p r c", c=NG)
            rbuf = arena[:, 0:R * NG].rearrange(
                "p (r c) -> p r c", c=NG)
            summ_h = summ_sb[:, 0:2 * R * TW].rearrange(
                "p (a r c) -> p a r c", a=2, r=R)

            def tree(rg):
                cur = hv[:, :, rg]
                nr = rg.stop - rg.start
                w = NB
                off = 0
                lvl = 0
                while w > TW:
                    w2 = w // 2
                    out_h = (summ_h[:, :, rg] if w2 == TW
                             else bufs_cycle[lvl % 2][:, :, rg, 0:w2])
                    nc.vector.tensor_tensor(
                        out=out_h, in0=cur[:, :, :, 0:w:2],
                        in1=cur[:, :, :, 1:w:2], op=op.min)
                    nc.vector.tensor_tensor(
                        out=scratch[:, rg, off:off + w2].unsqueeze(1),
                        in0=cur[:, 0:1, :, 1:w:2],
                        in1=cur[:, 1:2, :, 0:w:2],
                        op=op.add)
                    off += w2
                    cur = out_h
                    w = w2
                    lvl += 1

            bufs_cycle = [treeB, treeC]
            tree(slice(0, 8))

            # ---- threshold 2 lands mid-tree; broadcast overlaps recip ----
            nc.vector.reciprocal(out=trec[0:1, 8:10], in_=arsb2[0:1, 0:2])
            nc.vector.scalar_tensor_tensor(
                out=tmean[0:1, 8:10], in0=trec[0:1, 8:10], scalar=0.5,
                op0=op.mult, in1=arsb2[0:1, 6:8], op1=op.mult)
            nc.vector.tensor_tensor(
                out=tsx[0:1, 8:10], in0=arsb2[0:1, 2:4],
                in1=arsb2[0:1, 4:6], op=op.add)
            nc.vector.scalar_tensor_tensor(
                out=tthr[0:1, 8:10], in0=tsx[0:1, 8:10], scalar=0.5 / L,
                op0=op.mult, in1=tmean[0:1, 8:10], op1=op.add)
            nc.tensor.matmul(
                out=psb2, lhsT=ones_b, rhs=tthr[0:1, 8:10],
                start=True, stop=True)
            nc.scalar.copy(tbc[:, 8:10], psb2)

            nc.vector.reciprocal_approx_fast(
                out=rbuf[:, 0:8], in_=scratch[:, 0:8])
            nc.vector.tensor_reduce(
                out=summ_sb[:, 2 * R * TW:2 * R * TW + 8], in_=rbuf[:, 0:8],
                axis=mybir.AxisListType.X, op=op.add)
            # rows 0-7 output columns fly while rows 8-9 finish
            summ_ap = summ.ap()
            for c0, c1 in ((0, 8 * TW), (R * TW, R * TW + 8 * TW),
                           (2 * R * TW, 2 * R * TW + 8),
                           (SUMW - R, SUMW - 2)):
                nc.sync.dma_start(summ_ap[:, c0:c1], summ_sb[:, c0:c1])

            # ---- phase B rows 8..9 ----
            for r_ in (8, 9):
                nc.vector.scalar_tensor_tensor(
                    out=notv[:, r_], scalar=0.0, op0=op.bypass,
                    in0=B4m89[:, (r_ - 8) * NB:(r_ - 7) * NB],
                    in1=tbc[:, r_:r_ + 1].broadcast_to([P, NB]),
                    op1=op.is_le,
                    accum_out=summ_sb[:, SUMW - R + r_:SUMW - R + r_ + 1])
            nc.vector.copy_predicated(
                out=hv[:, :, 8:10],
                mask=notv[:, 8:10].bitcast(mybir.dt.int32).unsqueeze(1)
                .broadcast_to([P, 2, 2, NB]),
                data=big.broadcast_to([P, 2, 2, NB]))
            tree(slice(8, 10))
            nc.vector.reciprocal_approx_fast(
                out=rbuf[:, 8:10], in_=scratch[:, 8:10])
            nc.vector.tensor_reduce(
                out=summ_sb[:, 2 * R * TW + 8:2 * R * TW + R],
                in_=rbuf[:, 8:10],
                axis=mybir.AxisListType.X, op=op.add)
            for c0, c1 in ((8 * TW, R * TW), ((R + 8) * TW, 2 * R * TW),
                           (2 * R * TW + 8, 2 * R * TW + R),
                           (SUMW - 2, SUMW)):
                nc.sync.dma_start(summ_ap[:, c0:c1], summ_sb[:, c0:c1])

    nc.compile()
    return nc


def _get_runner():
    """Build once; return fn(in_maps) -> list of per-core {name: np.ndarray}."""
    if "runner" in _STATE:
        return _STATE["runner"]

    import jax
    import jax.numpy as jnp
    from jax.sharding import Mesh, PartitionSpec
    from jax.experimental.shard_map import shard_map
    from concourse import bass2jax, mybir

    nc = _build_program()
    bass2jax.install_neuronx_cc_hook()

    partition_name = (
        nc.partition_id_tensor.name if nc.partition_id_tensor else None
    )
    in_names, out_names, out_avals, zero_outs = [], [], [], []
    for alloc in nc.m.functions[0].allocations:
        if not isinstance(alloc, mybir.MemoryLocationSet):
            continue
        name = alloc.memorylocations[0].name
        if alloc.kind == "ExternalInput":
            if name != partition_name:
                in_names.append(name)
        elif alloc.kind == "ExternalOutput":
            out_names.append(name)
            shape = tuple(alloc.tensor_shape)
            dtype = mybir.dt.np(alloc.dtype)
            out_avals.append(jax.core.ShapedArray(shape, dtype))
            zero_outs.append(np.zeros(shape, dtype))
    n_params = len(in_names)
    n_outs = len(out_avals)
    all_names = in_names + out_names
    if partition_name is not None:
        all_names = all_names + [partition_name]

    def _body(*args):
        operands = list(args)
        if partition_name is not None:
            operands.append(bass2jax.partition_id_tensor())
        outs = bass2jax._bass_exec_p.bind(
            *operands,
            out_avals=tuple(out_avals),
            in_names=tuple(all_names),
            out_names=tuple(out_names),
            lowering_input_output_aliases=(),
            sim_require_finite=False,
            sim_require_nnan=False,
            nc=nc,
        )
        return tuple(outs)

    devices = jax.devices()[:C]
    assert len(devices) == C, f"need {C} devices, have {len(jax.devices())}"
    mesh = Mesh(np.asarray(devices), ("core",))
    donate = tuple(range(n_params, n_params + n_outs))
    sharded = jax.jit(
        shard_map(
            _body, mesh=mesh,
            in_specs=(PartitionSpec("core"),) * (n_params + n_outs),
            out_specs=(PartitionSpec("core"),) * n_outs,
            check_rep=False,
        ),
        donate_argnums=donate,
        keep_unused=True,
    )

    def run(in_maps):
        concat_in = [
            np.concatenate([np.asarray(m[nm]) for m in in_maps], axis=0)
            for nm in in_names
        ]
        concat_zeros = [
            np.zeros((C * z.shape[0], *z.shape[1:]), z.dtype) for z in zero_outs
        ]
        out_arrs = sharded(*concat_in, *concat_zeros)
        return [
            {nm: np.asarray(out_arrs[i]).reshape(C, *out_avals[i].shape)[c]
             for i, nm in enumerate(out_names)}
            for c in range(C)
        ]

    run.in_names = in_names
    run.out_names = out_names
    run.sharded = sharded
    run.zero_outs = zero_outs
    _STATE["runner"] = run
    return run


def make_in_maps(rppg, ppg):
    sigs = np.concatenate(
        [np.asarray(ppg, np.float32).reshape(N, L),
         np.asarray(rppg, np.float32).reshape(N, L)], axis=0)
    padded = np.full((R, L + 10), -np.inf, np.float32)
    padded[:, 5:5 + L] = sigs
    win = np.lib.stride_tricks.sliding_window_view(padded, TILE_W, axis=1)
    in_maps = []
    prow = np.arange(P, dtype=np.float32)[:, None] * PW
    brow = np.arange(NB, dtype=np.float32)[None, :] * 4.0
    for c in range(C):
        xin_c = np.ascontiguousarray(win[:, c * SEG:c * SEG + SEG:PW, :])
        bidx_c = (np.float32(c * SEG) + prow + brow).astype(np.float32)
        in_maps.append({"xin": xin_c, "bidx": bidx_c})
    return in_maps


def stitch(results, fs):
    summ = np.stack([results[c]["summ"] for c in range(C)])  # [C, 128, 340]
    hr = np.zeros(R)
    for r in range(R):
        # sub-strips of TW blocks: [C, P, TW] in global time order
        f = summ[:, :, r * TW:(r + 1) * TW].reshape(-1).astype(np.float64)
        g = -summ[:, :, (R + r) * TW:(R + r + 1) * TW].reshape(-1).astype(
            np.float64)
        s = summ[:, :, 2 * R * TW + r].astype(np.float64).sum()
        n = (512.0 - summ[:, :, 2 * R * TW + R + r].astype(np.float64)).sum()
        ne = f < float(BIG) / 2
        fs_, gs_ = f[ne], g[ne]
        s += (1.0 / (fs_[1:] - gs_[:-1])).sum()
        hr[r] = 60.0 * float(fs) * s / (n - 1.0)
    return np.float32(np.mean(np.abs(hr[0:N] - hr[N:R]) / hr[0:N]))


def kernel(rppg, ppg, fs, epoch):
    run = _get_runner()
    results = run(make_in_maps(rppg, ppg))
    return stitch(results, fs)
